# revision 14
# baseline (speedup 1.0000x reference)
"""Trainium2 Bass kernel for nn_ClusterLoss (vq_codebook).

reference:
    f = l2norm(features); c = l2norm(centers)
    sims = f @ c.T ; a = argmax(sims, -1)
    loss = mean(sum((f - centers[a])**2, -1))

Device algorithm (per core, data-parallel over N, 16384 rows each),
3-hop pipeline PE -> DVE -> ACT per 128-row tile:
  PE : H = lnr_bcast + beta*(f_bf16 @ c_hat_bf16.T)   (PSUM, one accum group:
       K=128 broadcast matmul pair preloads ln||c_k||, then the 2 main
       matmuls accumulate G; K=1 rank-1 matmuls are ~3x slower on HW, so the
       lnr row is injected via (ones/128) @ lnr_bcast instead)
  DVE: custom op NEG_SUB_MIN_REDUCE (see dve_ext):
       negbm = min_k(lnr_k - H_k) = -max_k G  (the exp bias), out = trash
  ACT: E = exp(H + negbm); S = sum_k E  (fused accum_out)
       = sum_k ||c_k||*e^{beta(cos_k - max cos)} ~= ||c_argmax||
  per-row loss = 1 - 2*(m/beta)*S/||f|| + S^2; ||f||^2 comes in as a
  host-computed input (exact, from the f32 features); host sums losses (f64).

Identity: ||f_hat - c_a||^2 = 1 - 2*cos*||c_a|| + ||c_a||^2; argmax of cos-sim
is invariant to scaling by beta/||f||, so raw bf16 features feed the matmul.
"""
import os
import sys

sys.path.insert(0, "/opt/trn_rl_repo")

_OPT = os.environ.get("KOPT", "lag2,gbufs4,stripes,hostnorm,chunk8,cdve")

from contextlib import ExitStack

import numpy as np

import concourse.bass as bass
import concourse.bacc as bacc
import concourse.mybir as mybir
from concourse import tile
from concourse.bass_utils import run_bass_kernel_spmd

# ---------------------------------------------------------------------------
# Custom DVE op NEG_SUB_MIN_REDUCE (registered into concourse.dve_ops):
#   out[k]    = in1[k] - in0[k]
#   accum_out = min(s0, min_k out[k])
# With in0 = H = G + lnr (PSUM f32) and in1 = LNR_bcast (SBUF):
# out = -G (trash), accum_out = -max_k G — the exp bias in one DVE pass,
# with no post-max PE work on the PSUM tile.
# ---------------------------------------------------------------------------
import concourse.dve_ops as _dve_ops
from concourse.dve_ops import DveOp as _DveOp
from concourse.dve_spec import C0 as _C0, Spec as _Spec, Src0 as _Src0, Src1 as _Src1, lower as _dve_lower, minn as _minn
from concourse.dve_uop import DveOpSpec as _DveOpSpec


def _nsmr_ref(in0, in1, c0, c1, c2):
    out = (in1.astype(np.float32) - in0.astype(np.float32)).astype(np.float32)
    acc = np.minimum(out.reshape(out.shape[0], -1).min(axis=-1, keepdims=True), c0)
    return out, acc


def _make_nsmr_op() -> _DveOp:
    name = "NEG_SUB_MIN_REDUCE"
    if name in _dve_ops._SUB_OPCODE_FOR_NAME:  # already registered (re-import)
        for op in _dve_ops.OPS:
            if op.name == name:
                return op
    spec = _Spec(body=_Src1 - _Src0, accum=_minn, accum_init=_C0, reference=_nsmr_ref)
    row = max(_dve_ops._SUB_OPCODE_FOR_NAME.values()) + 1
    assert row < 0x20, "no free custom-DVE rows"
    shas = {}
    for ver in ("v3", "v4"):
        s = _DveOpSpec(name=name, opcode=row, uops=_dve_lower(spec, ver=ver), rd1_en=True)
        shas[ver] = s.sha(ver)
    op = _DveOp(name, spec, subdim=False, uops_sha=shas)
    _dve_ops.OPS.append(op)
    _dve_ops._SUB_OPCODE_FOR_NAME[name] = row
    _dve_ops.CUSTOM_DVE_SPECS[name] = spec
    return op


_NSMR = _make_nsmr_op()


def _neg_sub_min_reduce(nc, out, in0, in1, accum_out):
    """accum_out = min(3e38, min_k (in1 - in0)); out = in1 - in0 (trash)."""
    return nc.vector._custom_dve(
        _NSMR, out=out, in0=in0, in1=in1, s0=3.0e38, accum_out=accum_out
    )

F32 = mybir.dt.float32
BF16 = mybir.dt.bfloat16
NP_BF16 = mybir.dt.np(mybir.dt.bfloat16)
AF = mybir.ActivationFunctionType
AX = mybir.AxisListType

N_CORES = 8
N_TOTAL = 131072
D = 128
K = 1024
ROWS_PER_CORE = N_TOTAL // N_CORES
BETA = 32768.0  # power of two: bf16(beta*c_hat) == beta*bf16(c_hat)

_nc_cache = {}


def build_nc(rows_per_core=ROWS_PER_CORE):
    return build_nc_rep(rows_per_core, rep=1)


def build_nc_rep(rows_per_core=ROWS_PER_CORE, rep=1):
    """Build + compile the per-core bass program (SPMD across 8 cores).

    rep>1 wraps the (idempotent) main loop in a hardware For_i loop --
    used for wall-clock HW timing with a constant-size NEFF. rep<0 unrolls
    |rep| passes in python (for TimelineSim steady-state checks).
    """
    if (rows_per_core, rep) in _nc_cache:
        return _nc_cache[(rows_per_core, rep)]

    R = rows_per_core
    T = R // 128  # number of 128-row tiles
    KT = K // 128  # center tiles (8)
    NCH = 8 if "chunk8" in _OPT else 1  # input DMA chunks
    TCH = T // NCH  # tiles per chunk
    CDVE = "cdve" in _OPT

    nc = bacc.Bacc("TRN2", target_bir_lowering=False, debug=False, num_devices=N_CORES)

    ft = nc.dram_tensor("ft", [128, R], BF16, kind="ExternalInput").ap()  # f^T
    nw_in = nc.dram_tensor("nw", [128, T], F32, kind="ExternalInput").ap()
    cn = nc.dram_tensor("cn", [K, 128], F32, kind="ExternalInput").ap()  # centers
    ident = nc.dram_tensor("ident", [128, 128], BF16, kind="ExternalInput").ap()
    m128 = nc.dram_tensor("m128", [128, 128], BF16, kind="ExternalInput").ap()
    ones1 = nc.dram_tensor("ones1", [1, 128], BF16, kind="ExternalInput").ap()
    lossw = nc.dram_tensor("lossw", [128, T], F32, kind="ExternalOutput").ap()

    STRIPES = 8 if "stripes" in _OPT else 0
    LAG = 2 if "lag2" in _OPT else 1

    with tile.TileContext(nc) as tc, ExitStack() as ctx:
        const = ctx.enter_context(tc.tile_pool(name="const", bufs=1))
        setup = ctx.enter_context(tc.tile_pool(name="setup", bufs=2))
        setup_ps_cm = tc.tile_pool(name="setup_ps", bufs=1, space="PSUM")
        setup_ps = setup_ps_cm.__enter__()
        trash = ctx.enter_context(tc.tile_pool(name="trash", bufs=3))

        # ---------------- constants / big input loads ----------------
        id_sb = const.tile([128, 128], BF16)
        nc.sync.dma_start(id_sb[:], ident)
        m128_sb = const.tile([128, 128], BF16)
        nc.sync.dma_start(m128_sb[:], m128)
        on_sb = const.tile([1, 128], BF16)
        nc.sync.dma_start(on_sb[:], ones1)

        ct_sb = const.tile([128, KT * 128], F32)  # natural centers [kpart, (j d)]
        ct_v = ct_sb[:].rearrange("p (j d) -> p j d", d=128)
        nc.sync.dma_start(ct_v, cn.rearrange("(j p) d -> p j d", p=128))

        n2w = const.tile([128, T], F32)
        nc.sync.dma_start(n2w[:], nw_in)

        # chunked feature loads (compute on chunk 0 can start immediately)
        ft_c = []
        for i in range(NCH):
            ftt = const.tile([128, TCH * 128], BF16, name=f"ftc{i}")
            nc.sync.dma_start(ftt[:], ft[:, i * TCH * 128 : (i + 1) * TCH * 128])
            ft_c.append(ftt)

        def ft_blk(t):
            return ft_c[t // TCH][:, (t % TCH) * 128 : (t % TCH + 1) * 128]

        # ---------------- center setup ----------------
        # q[p, j] = ||c_{j*128+p}||^2
        qw = setup.tile([128, KT], F32)
        for j in range(KT):
            sq_t = trash.tile([128, 128], F32, tag="sq_t")
            nc.scalar.activation(
                sq_t[:], ct_v[:, j, :], AF.Square, accum_out=qw[:, j : j + 1]
            )
        # rinv = 1/||c||  (reciprocal then sqrt)
        qinv = setup.tile([128, KT], F32)
        nc.vector.reciprocal(qinv[:], qw[:])
        rinv = setup.tile([128, KT], F32)
        nc.scalar.activation(rinv[:], qinv[:], AF.Sqrt)
        # r = q * rinv = ||c||;  lnr = ln ||c||
        rr = setup.tile([128, KT], F32)
        nc.vector.tensor_mul(rr[:], qw[:], rinv[:])
        lnr = setup.tile([128, KT], F32)
        nc.scalar.activation(lnr[:], rr[:], AF.Ln)
        lnr_bf = setup.tile([128, KT], BF16)
        nc.vector.tensor_copy(lnr_bf[:], lnr[:])
        # scaled normalizer: beta / ||c||
        rinv_b = setup.tile([128, KT], F32)
        nc.vector.tensor_scalar_mul(rinv_b[:], rinv[:], float(BETA))

        # c_hat_scaled tiles (natural layout) then PE-transpose into chT [d, k]
        chT_sb = const.tile([128, K], BF16)
        for j in range(KT):
            ch_j = setup.tile([128, 128], BF16, tag="ch_j")
            nc.vector.tensor_scalar_mul(ch_j[:], ct_v[:, j, :], rinv_b[:, j : j + 1])
            chT_ps = setup_ps.tile([128, 128], BF16, tag="chT_ps")
            nc.tensor.transpose(chT_ps[:], ch_j[:], id_sb[:])
            nc.scalar.activation(
                chT_sb[:, j * 128 : (j + 1) * 128], chT_ps[:], AF.Copy
            )

        # lnr_row [1, K] at partition 0 via column-wise PE transposes
        lnr_row = setup.tile([1, K], BF16)
        for j in range(KT):
            rt_ps = setup_ps.tile([1, 128], BF16, tag="rt_ps")
            nc.tensor.transpose(rt_ps[:], lnr_bf[:, j : j + 1], id_sb[:])
            nc.scalar.activation(
                lnr_row[:, j * 128 : (j + 1) * 128], rt_ps[:], AF.Copy
            )

        # LNR_bcast [128, K] bf16: lnr_row broadcast down all partitions
        # (K=1 matmul pair, setup-only), copied out through ACT.
        lnr_bc = const.tile([128, K], BF16)
        for h in range(2):
            bc_ps = setup_ps.tile([128, 512], F32, tag="bc_ps")
            nc.tensor.matmul(
                bc_ps[:], on_sb[:], lnr_row[:, h * 512 : (h + 1) * 512],
                start=True, stop=True,
            )
            nc.scalar.activation(
                lnr_bc[:, h * 512 : (h + 1) * 512], bc_ps[:], AF.Copy
            )

        setup_ps_cm.__exit__(None, None, None)
        gpool = ctx.enter_context(tc.tile_pool(
            name="gpool", bufs=4 if "gbufs4" in _OPT else 3, space="PSUM"))
        epool = ctx.enter_context(tc.tile_pool(name="epool", bufs=3))
        dpool = ctx.enter_context(tc.tile_pool(name="dpool", bufs=3))

        # ---------------- wide accumulators ----------------
        sw = const.tile([128, T], F32)
        if STRIPES:
            negbm_s = [
                const.tile([128, T // STRIPES], F32, name=f"nbm{j}", tag=f"nbm{j}")
                for j in range(STRIPES)
            ]

            def negbm_col(t):
                return negbm_s[t % STRIPES][:, t // STRIPES : t // STRIPES + 1]
        else:
            negbm_w0 = const.tile([128, T], F32)

            def negbm_col(t):
                return negbm_w0[:, t : t + 1]

        # ---------------- main loop (software pipeline) ----------------
        def emit_head(t):
            g_ps = gpool.tile([128, K], F32)
            lhs = ft_blk(t)
            if CDVE:
                # preload lnr (K=128 broadcast matmul), then accumulate G
                nc.tensor.matmul(g_ps[:, 0:512], m128_sb[:], lnr_bc[:, 0:512], start=True, stop=False, skip_group_check=True)
                nc.tensor.matmul(g_ps[:, 512:1024], m128_sb[:], lnr_bc[:, 512:1024], start=True, stop=False, skip_group_check=True)
                nc.tensor.matmul(g_ps[:, 0:512], lhs, chT_sb[:, 0:512], start=False, stop=True, skip_group_check=True)
                nc.tensor.matmul(g_ps[:, 512:1024], lhs, chT_sb[:, 512:1024], start=False, stop=True, skip_group_check=True)
                bias_ap = negbm_col(t)
                dout = dpool.tile([128, K], BF16, tag="dout")
                _neg_sub_min_reduce(nc, dout[:], g_ps[:], lnr_bc[:], bias_ap)
            else:
                nc.tensor.matmul(g_ps[:, 0:512], lhs, chT_sb[:, 0:512], start=True, stop=True)
                nc.tensor.matmul(g_ps[:, 512:1024], lhs, chT_sb[:, 512:1024], start=True, stop=True)
                bias_ap = negbm_col(t)
                nc.vector.reduce_max(bias_ap, g_ps[:], axis=AX.X, negate=True)
            return g_ps, bias_ap

        def emit_tail(t, g_ps, bias_ap):
            if not CDVE:
                # lnr add via K=128 broadcast matmul (K=1 rank-1s are ~3x slower)
                nc.tensor.matmul(g_ps[:, 0:512], m128_sb[:], lnr_bc[:, 0:512], start=False, stop=True, skip_group_check=True)
                nc.tensor.matmul(g_ps[:, 512:1024], m128_sb[:], lnr_bc[:, 512:1024], start=False, stop=True, skip_group_check=True)
            e_sb = epool.tile([128, K], BF16)
            nc.scalar.activation(
                e_sb[:], g_ps[:], AF.Exp,
                bias=bias_ap, scale=1.0,
                accum_out=sw[:, t : t + 1],
            )

        def one_pass(_i=None):
            from collections import deque

            pend = deque()
            for t in range(T):
                g_ps, bias_ap = emit_head(t)
                pend.append((t, g_ps, bias_ap))
                if len(pend) > LAG:
                    emit_tail(*pend.popleft())
            while pend:
                emit_tail(*pend.popleft())

        if rep == 1:
            one_pass()
        elif rep < 0:  # python-unrolled (for TimelineSim steady-state checks)
            for _ in range(-rep):
                one_pass()
        else:
            with tc.For_i(0, rep) as _i:
                one_pass(_i)

        # ---------------- epilogue: per-row loss ----------------
        if STRIPES:
            negbm_w = setup.tile([128, T], F32)
            nv = negbm_w[:].rearrange("p (c j) -> p c j", j=STRIPES)
            for j in range(STRIPES):
                nc.vector.tensor_copy(nv[:, :, j], negbm_s[j][:])
        else:
            negbm_w = negbm_w0
        m_w = setup.tile([128, T], F32)
        nc.vector.tensor_scalar_mul(m_w[:], negbm_w[:], -1.0 / BETA)
        n2i = setup.tile([128, T], F32)
        nc.vector.reciprocal(n2i[:], n2w[:])
        invn = setup.tile([128, T], F32)
        nc.scalar.activation(invn[:], n2i[:], AF.Sqrt)
        a_w = setup.tile([128, T], F32)
        nc.vector.tensor_mul(a_w[:], m_w[:], invn[:])
        b_w = setup.tile([128, T], F32)
        nc.vector.tensor_mul(b_w[:], a_w[:], sw[:])
        b2_w = setup.tile([128, T], F32)
        nc.vector.tensor_scalar_mul(b2_w[:], b_w[:], -2.0)
        r2_w = setup.tile([128, T], F32)
        nc.vector.tensor_mul(r2_w[:], sw[:], sw[:])
        t3_w = setup.tile([128, T], F32)
        nc.vector.tensor_add(t3_w[:], r2_w[:], b2_w[:])
        lw = setup.tile([128, T], F32)
        nc.vector.tensor_scalar_add(lw[:], t3_w[:], 1.0)
        nc.sync.dma_start(lossw, lw[:])

    nc.compile()
    _nc_cache[(rows_per_core, rep)] = nc
    return nc


def make_in_maps(features, centers, rows_per_core=ROWS_PER_CORE, n_cores=N_CORES):
    f_bf = features.astype(NP_BF16)
    shards = f_bf.reshape(n_cores, rows_per_core, D)
    ident = np.eye(128, dtype=NP_BF16)
    m128 = np.full((128, 128), 1.0 / 128.0, dtype=NP_BF16)
    ones1 = np.ones((1, 128), dtype=NP_BF16)
    cns = np.ascontiguousarray(centers.astype(np.float32))
    # per-row ||f||^2 from the f32 features (exact), laid out [128, T]
    n2 = np.square(features.astype(np.float32)).sum(axis=1)
    n2_shards = n2.reshape(n_cores, rows_per_core // 128, 128)
    in_maps = []
    for c in range(n_cores):
        s = shards[c]
        in_maps.append(
            {
                "ft": np.ascontiguousarray(s.T),
                "nw": np.ascontiguousarray(n2_shards[c].T),
                "cn": cns,
                "ident": ident,
                "m128": m128,
                "ones1": ones1,
            }
        )
    return in_maps


def kernel(features, centers):
    features = np.asarray(features)
    centers = np.asarray(centers)
    nc = build_nc(ROWS_PER_CORE)
    in_maps = make_in_maps(features, centers)
    res = run_bass_kernel_spmd(nc, in_maps, core_ids=list(range(N_CORES)))
    total = 0.0
    for c in range(N_CORES):
        total += res.results[c]["lossw"].astype(np.float64).sum()
    return np.float32(total / (ROWS_PER_CORE * N_CORES))
